# revision 7
# baseline (speedup 1.0000x reference)
"""Mamba selective-scan kernel for Trainium2 (Bass/Tile), 8-core SPMD.

Problem: y[b,d,t] = sum_n C[b,n,t] * x[b,d,n,t] + D[d]*u[b,d,t]
         x[.,t]   = exp(dt*A) * x[.,t-1] + dt*u*B[b,n,t]
         dt       = softplus(delta + delta_bias)
Shapes: u/delta [4,1536,4096], B/C [4,16,4096], A [1536,16], D/delta_bias [1536].

Sharding: core c -> batch b=c//2, dim half h=c%2 (768 channels each).

Per core (V2):
- 6 channel-tiles of 128 partitions; seq in quarters of 1024, state handoff
  through [128,16] bf16 states tiles.
- softplus(z) = -ln(sigmoid(-z)) on ScalarE (Sigmoid then Ln); sign folded
  into Aneg (exp scale) and the dtu scalar_tensor_tensor.
- time recurrence: hardware tensor_tensor_scan on VectorE (dA f32, dBu bf16).
- n-sum accumulated on TensorE: PSUM seeded with diag(D) @ u, then
  identity(bf16) @ (x*C) accumulation matmuls; ct-groups of 3 (PSUM capacity).
- B/C rows broadcast across partitions via stride-0 DMA as bf16.
"""

import ml_dtypes
import numpy as np

import concourse.bacc as bacc
import concourse.mybir as mybir
import concourse.tile as tile
from concourse import bass
from concourse.bass_utils import run_bass_kernel_spmd

F32 = mybir.dt.float32
BF16 = mybir.dt.bfloat16
ALU = mybir.AluOpType
ACTF = mybir.ActivationFunctionType

BATCH, DIM, SEQ, DSTATE = 4, 1536, 4096, 16
N_CORES = 8
DL = DIM // 2          # channels per core
N_CT = DL // 128       # channel tiles per core
CT_GROUP = 3           # channel tiles per PSUM group


def build_nc(dl=DL, seq=SEQ, dstate=DSTATE, s_chunk=1024, reps=1):
    """Build the single-core Bass program (same NEFF runs on all 8 cores)."""
    n_ct = dl // 128
    n_q = seq // s_chunk
    half = s_chunk // 2
    groups = [list(range(g, min(g + CT_GROUP, n_ct)))
              for g in range(0, n_ct, CT_GROUP)]
    nc = bacc.Bacc("TRN2", target_bir_lowering=False, debug=False)

    u_d = nc.dram_tensor("u", [dl, seq], F32, kind="ExternalInput").ap()
    delta_d = nc.dram_tensor("delta", [dl, seq], F32, kind="ExternalInput").ap()
    B_d = nc.dram_tensor("Bmat", [dstate, seq], BF16, kind="ExternalInput").ap()
    C_d = nc.dram_tensor("Cmat", [dstate, seq], BF16, kind="ExternalInput").ap()
    A_d = nc.dram_tensor("Amat", [dl, dstate], F32, kind="ExternalInput").ap()
    D_d = nc.dram_tensor("Dvec", [dl, 1], F32, kind="ExternalInput").ap()
    db_d = nc.dram_tensor("dbias", [dl, 1], F32, kind="ExternalInput").ap()
    eye_d = nc.dram_tensor("eye", [128, 128], F32, kind="ExternalInput").ap()
    y_d = nc.dram_tensor("y", [dl, seq], F32, kind="ExternalOutput").ap()

    with tile.TileContext(nc) as tc:
        with (
            tc.tile_pool(name="consts", bufs=1) as consts,
            tc.tile_pool(name="states", bufs=1) as states_pool,
            tc.tile_pool(name="io", bufs=2) as io,
            tc.tile_pool(name="useed", bufs=n_ct) as useed,
            tc.tile_pool(name="resid", bufs=1) as resid,
            tc.tile_pool(name="bc", bufs=3) as bcp,
            tc.tile_pool(name="work", bufs=3) as work,
            tc.tile_pool(name="yout", bufs=2) as yout,
            tc.tile_pool(name="psum", bufs=1, space="PSUM") as psp,
        ):
            eye = consts.tile([128, 128], F32, tag="eye", name="eye")
            nc.sync.dma_start(eye[:], eye_d[:, :])
            eyeb = consts.tile([128, 128], BF16, tag="eyeb", name="eyeb")
            nc.vector.tensor_copy(eyeb[:], eye[:])

            A_sb, db_sb, diag_sb, states = [], [], [], []
            for ct in range(n_ct):
                rows = slice(ct * 128, (ct + 1) * 128)
                a_raw = consts.tile([128, dstate], F32, tag=f"Ar{ct}",
                                    name=f"A_raw{ct}")
                nc.sync.dma_start(a_raw[:], A_d[rows, :])
                a_t = consts.tile([128, dstate], F32, tag=f"A{ct}",
                                  name=f"A_sb{ct}")
                nc.vector.tensor_scalar(a_t[:], a_raw[:], -1.0, None,
                                        op0=ALU.mult)
                A_sb.append(a_t)
                b_raw = consts.tile([128, 1], F32, tag=f"dbr{ct}",
                                    name=f"db_raw{ct}")
                nc.sync.dma_start(b_raw[:], db_d[rows, :])
                b_t = consts.tile([128, 1], F32, tag=f"db{ct}",
                                  name=f"db_sb{ct}")
                nc.vector.tensor_scalar(b_t[:], b_raw[:], -1.0, None,
                                        op0=ALU.mult)
                db_sb.append(b_t)
                d_t = consts.tile([128, 1], F32, tag=f"D{ct}", name=f"D_sb{ct}")
                nc.sync.dma_start(d_t[:], D_d[rows, :])
                dg = consts.tile([128, 128], F32, tag=f"dg{ct}",
                                 name=f"diag{ct}")
                nc.vector.tensor_scalar(dg[:], eye[:], d_t[:, 0:1], None,
                                        op0=ALU.mult)
                diag_sb.append(dg)
                s_t = states_pool.tile([128, dstate], BF16, tag=f"st{ct}",
                                       name=f"states{ct}")
                states.append(s_t)

            for rep in range(reps):
              for q in range(n_q):
                lo, hi = q * s_chunk, (q + 1) * s_chunk
                dt_all = [None] * n_ct
                dtu_all = [None] * n_ct
                u_all = [None] * n_ct
                # Phase A: dt = softplus, dtu = dt*u (bf16)
                for ct in range(n_ct):
                    rows = slice(ct * 128, (ct + 1) * 128)
                    u_t = useed.tile([128, s_chunk], F32, tag="u",
                                     name=f"u_t{ct}_{q}")
                    nc.sync.dma_start(u_t[:], u_d[rows, lo:hi])
                    de_t = io.tile([128, s_chunk], F32, tag="de",
                                   name=f"de_t{ct}_{q}")
                    nc.sync.dma_start(de_t[:], delta_d[rows, lo:hi])
                    sg_t = io.tile([128, s_chunk], F32, tag="sg",
                                   name=f"sg_t{ct}_{q}")
                    nc.scalar.activation(sg_t[:], de_t[:], ACTF.Sigmoid,
                                         bias=db_sb[ct][:, 0:1], scale=-1.0)
                    dt_t = resid.tile([128, s_chunk], F32, tag=f"dt{ct}",
                                      name=f"dt_t{ct}_{q}")
                    nc.scalar.activation(dt_t[:], sg_t[:], ACTF.Ln,
                                         bias=0.0, scale=1.0)
                    dtu_t = resid.tile([128, s_chunk], BF16, tag=f"dtu{ct}",
                                       name=f"dtu_t{ct}_{q}")
                    nc.vector.scalar_tensor_tensor(dtu_t[:], dt_t[:], -1.0,
                                                   u_t[:], op0=ALU.mult,
                                                   op1=ALU.mult)
                    dt_all[ct] = dt_t
                    dtu_all[ct] = dtu_t
                    u_all[ct] = u_t

                for grp in groups:
                    # Seed PSUM with D*u via diag(D) matmuls
                    ps_t = {}
                    for ct in grp:
                        for h in range(2):
                            ps = psp.tile([128, half], F32,
                                          tag=f"ps{ct % CT_GROUP}_{h}",
                                          name=f"ps{ct}_{q}_{h}")
                            ps_t[(ct, h)] = ps
                            cols = slice(h * half, (h + 1) * half)
                            nc.tensor.matmul(ps[:], diag_sb[ct][:],
                                             u_all[ct][:, cols],
                                             start=True, stop=False)
                    for n in range(dstate):
                        Bbc = bcp.tile([128, s_chunk], BF16, tag="Bbc",
                                       name=f"Bbc{n}_{q}_{grp[0]}")
                        nc.sync.dma_start(
                            Bbc[:],
                            B_d[n:n + 1, lo:hi].broadcast_to([128, s_chunk]))
                        Cbc = bcp.tile([128, s_chunk], BF16, tag="Cbc",
                                       name=f"Cbc{n}_{q}_{grp[0]}")
                        nc.sync.dma_start(
                            Cbc[:],
                            C_d[n:n + 1, lo:hi].broadcast_to([128, s_chunk]))
                        for ct in grp:
                            dA = work.tile([128, s_chunk], F32, tag="dA",
                                           name=f"dA{n}_{ct}_{q}")
                            nc.scalar.activation(dA[:], dt_all[ct][:],
                                                 ACTF.Exp, bias=0.0,
                                                 scale=A_sb[ct][:, n:n + 1])
                            dBu = work.tile([128, s_chunk], BF16, tag="dBu",
                                            name=f"dBu{n}_{ct}_{q}")
                            nc.vector.tensor_tensor(dBu[:], dtu_all[ct][:],
                                                    Bbc[:], op=ALU.mult)
                            x = work.tile([128, s_chunk], BF16, tag="x",
                                          name=f"x{n}_{ct}_{q}")
                            init = (0.0 if q == 0
                                    else states[ct][:, n:n + 1])
                            nc.vector.tensor_tensor_scan(x[:], dA[:], dBu[:],
                                                         init, op0=ALU.mult,
                                                         op1=ALU.add)
                            if q < n_q - 1:
                                nc.vector.tensor_copy(
                                    states[ct][:, n:n + 1],
                                    x[:, s_chunk - 1:s_chunk])
                            xC = work.tile([128, s_chunk], BF16, tag="xC",
                                           name=f"xC{n}_{ct}_{q}")
                            nc.vector.tensor_tensor(xC[:], x[:], Cbc[:],
                                                    op=ALU.mult)
                            last = n == dstate - 1
                            for h in range(2):
                                cols = slice(h * half, (h + 1) * half)
                                nc.tensor.matmul(ps_t[(ct, h)][:], eyeb[:],
                                                 xC[:, cols], start=False,
                                                 stop=last)
                    # Evict PSUM -> SBUF -> DRAM
                    for ct in grp:
                        rows = slice(ct * 128, (ct + 1) * 128)
                        yo = yout.tile([128, s_chunk], F32, tag="yo",
                                       name=f"yo{ct}_{q}")
                        for h in range(2):
                            cols = slice(h * half, (h + 1) * half)
                            nc.scalar.copy(yo[:, cols], ps_t[(ct, h)][:])
                        nc.sync.dma_start(y_d[rows, lo:hi], yo[:])

    nc.compile()
    return nc


_NC_CACHE = {}


def _get_nc():
    if "nc" not in _NC_CACHE:
        _NC_CACHE["nc"] = build_nc()
    return _NC_CACHE["nc"]


def shard_inputs(u, delta, B, C, A, D, delta_bias):
    eye = np.eye(128, dtype=np.float32)
    in_maps = []
    for c in range(N_CORES):
        b, h = divmod(c, 2)
        sl = slice(h * DL, (h + 1) * DL)
        in_maps.append({
            "u": np.ascontiguousarray(u[b, sl]),
            "delta": np.ascontiguousarray(delta[b, sl]),
            "Bmat": np.ascontiguousarray(B[b]).astype(ml_dtypes.bfloat16),
            "Cmat": np.ascontiguousarray(C[b]).astype(ml_dtypes.bfloat16),
            "Amat": np.ascontiguousarray(A[sl]),
            "Dvec": np.ascontiguousarray(D[sl]).reshape(DL, 1),
            "dbias": np.ascontiguousarray(delta_bias[sl]).reshape(DL, 1),
            "eye": eye,
        })
    return in_maps


def kernel(u, delta, B, C, A, D, delta_bias, trace=False):
    u = np.asarray(u, np.float32)
    delta = np.asarray(delta, np.float32)
    B = np.asarray(B, np.float32)
    C = np.asarray(C, np.float32)
    A = np.asarray(A, np.float32)
    D = np.asarray(D, np.float32)
    delta_bias = np.asarray(delta_bias, np.float32)

    nc = _get_nc()
    in_maps = shard_inputs(u, delta, B, C, A, D, delta_bias)
    res = run_bass_kernel_spmd(nc, in_maps, core_ids=list(range(N_CORES)),
                               trace=trace)
    y = np.empty((BATCH, DIM, SEQ), np.float32)
    for c, r in enumerate(res.results):
        b, h = divmod(c, 2)
        y[b, h * DL:(h + 1) * DL] = r["y"]
    if trace:
        return y, res
    return y


# revision 8
# speedup vs baseline: 2.4396x; 2.4396x over previous
"""Mamba selective-scan kernel for Trainium2 (Bass/Tile), 8-core SPMD.

Problem: y[b,d,t] = sum_n C[b,n,t] * x[b,d,n,t] + D[d]*u[b,d,t]
         x[.,t]   = exp(dt*A) * x[.,t-1] + dt*u*B[b,n,t]
         dt       = softplus(delta + delta_bias)
Shapes: u/delta [4,1536,4096], B/C [4,16,4096], A [1536,16], D/delta_bias [1536].

Sharding: core c -> batch b=c//2, dim half h=c%2 (768 channels each).

Per core (V2):
- 6 channel-tiles of 128 partitions; seq in quarters of 1024, state handoff
  through [128,16] bf16 states tiles.
- softplus(z) = -ln(sigmoid(-z)) on ScalarE (Sigmoid then Ln); sign folded
  into Aneg (exp scale) and the dtu scalar_tensor_tensor.
- time recurrence: hardware tensor_tensor_scan on VectorE (dA f32, dBu bf16).
- n-sum accumulated on TensorE: PSUM seeded with diag(D) @ u, then
  identity(bf16) @ (x*C) accumulation matmuls; ct-groups of 3 (PSUM capacity).
- B/C rows broadcast across partitions via stride-0 DMA as bf16.
"""

import ml_dtypes
import numpy as np

import concourse.bacc as bacc
import concourse.mybir as mybir
import concourse.tile as tile
from concourse import bass
from concourse.bass_utils import run_bass_kernel_spmd

F32 = mybir.dt.float32
BF16 = mybir.dt.bfloat16
ALU = mybir.AluOpType
ACTF = mybir.ActivationFunctionType

BATCH, DIM, SEQ, DSTATE = 4, 1536, 4096, 16
N_CORES = 8
DL = DIM // 2          # channels per core
N_CT = DL // 128       # channel tiles per core
CT_GROUP = 3           # channel tiles per PSUM group


def build_nc(dl=DL, seq=SEQ, dstate=DSTATE, s_chunk=1024, reps=1):
    """Build the single-core Bass program (same NEFF runs on all 8 cores)."""
    n_ct = dl // 128
    n_q = seq // s_chunk
    half = s_chunk // 2
    groups = [list(range(g, min(g + CT_GROUP, n_ct)))
              for g in range(0, n_ct, CT_GROUP)]
    nc = bacc.Bacc("TRN2", target_bir_lowering=False, debug=False)

    u_d = nc.dram_tensor("u", [dl, seq], F32, kind="ExternalInput").ap()
    delta_d = nc.dram_tensor("delta", [dl, seq], F32, kind="ExternalInput").ap()
    B_d = nc.dram_tensor("Bmat", [dstate, seq], BF16, kind="ExternalInput").ap()
    C_d = nc.dram_tensor("Cmat", [dstate, seq], BF16, kind="ExternalInput").ap()
    A_d = nc.dram_tensor("Amat", [dl, dstate], F32, kind="ExternalInput").ap()
    D_d = nc.dram_tensor("Dvec", [dl, 1], F32, kind="ExternalInput").ap()
    db_d = nc.dram_tensor("dbias", [dl, 1], F32, kind="ExternalInput").ap()
    eye_d = nc.dram_tensor("eye", [128, 128], F32, kind="ExternalInput").ap()
    y_d = nc.dram_tensor("y", [dl, seq], F32, kind="ExternalOutput").ap()

    with tile.TileContext(nc) as tc:
        with (
            tc.tile_pool(name="consts", bufs=1) as consts,
            tc.tile_pool(name="states", bufs=1) as states_pool,
            tc.tile_pool(name="io", bufs=2) as io,
            tc.tile_pool(name="useed", bufs=n_ct) as useed,
            tc.tile_pool(name="resid", bufs=1) as resid,
            tc.tile_pool(name="bc", bufs=3) as bcp,
            tc.tile_pool(name="work", bufs=3) as work,
            tc.tile_pool(name="yout", bufs=2) as yout,
            tc.tile_pool(name="psum", bufs=1, space="PSUM") as psp,
        ):
            eye = consts.tile([128, 128], F32, tag="eye", name="eye")
            nc.sync.dma_start(eye[:], eye_d[:, :])
            eyeb = consts.tile([128, 128], BF16, tag="eyeb", name="eyeb")
            nc.vector.tensor_copy(eyeb[:], eye[:])

            A_sb, db_sb, diag_sb, states = [], [], [], []
            for ct in range(n_ct):
                rows = slice(ct * 128, (ct + 1) * 128)
                a_t = consts.tile([128, dstate], F32, tag=f"A{ct}",
                                  name=f"A_sb{ct}")
                nc.sync.dma_start(a_t[:], A_d[rows, :])
                A_sb.append(a_t)
                b_t = consts.tile([128, 1], F32, tag=f"db{ct}",
                                  name=f"db_sb{ct}")
                nc.sync.dma_start(b_t[:], db_d[rows, :])
                db_sb.append(b_t)
                d_t = consts.tile([128, 1], F32, tag=f"D{ct}", name=f"D_sb{ct}")
                nc.sync.dma_start(d_t[:], D_d[rows, :])
                dg = consts.tile([128, 128], F32, tag=f"dg{ct}",
                                 name=f"diag{ct}")
                nc.vector.tensor_scalar(dg[:], eye[:], d_t[:, 0:1], None,
                                        op0=ALU.mult)
                diag_sb.append(dg)
                s_t = states_pool.tile([128, dstate], BF16, tag=f"st{ct}",
                                       name=f"states{ct}")
                states.append(s_t)

            for rep in range(reps):
              for q in range(n_q):
                lo, hi = q * s_chunk, (q + 1) * s_chunk
                dt_all = [None] * n_ct
                dtu_all = [None] * n_ct
                u_all = [None] * n_ct
                # Phase A: dt = softplus, dtu = dt*u (bf16)
                for ct in range(n_ct):
                    rows = slice(ct * 128, (ct + 1) * 128)
                    u_t = useed.tile([128, s_chunk], F32, tag="u",
                                     name=f"u_t{ct}_{q}")
                    nc.sync.dma_start(u_t[:], u_d[rows, lo:hi])
                    de_t = io.tile([128, s_chunk], F32, tag="de",
                                   name=f"de_t{ct}_{q}")
                    nc.sync.dma_start(de_t[:], delta_d[rows, lo:hi])
                    sg_t = io.tile([128, s_chunk], F32, tag="sg",
                                   name=f"sg_t{ct}_{q}")
                    nc.scalar.activation(sg_t[:], de_t[:], ACTF.Exp,
                                         bias=db_sb[ct][:, 0:1], scale=1.0)
                    dt_t = resid.tile([128, s_chunk], F32, tag=f"dt{ct}",
                                      name=f"dt_t{ct}_{q}")
                    nc.scalar.activation(dt_t[:], sg_t[:], ACTF.Ln,
                                         bias=1.0, scale=1.0)
                    dtu_t = resid.tile([128, s_chunk], BF16, tag=f"dtu{ct}",
                                       name=f"dtu_t{ct}_{q}")
                    nc.vector.tensor_tensor(dtu_t[:], dt_t[:], u_t[:],
                                            op=ALU.mult)
                    dt_all[ct] = dt_t
                    dtu_all[ct] = dtu_t
                    u_all[ct] = u_t

                for grp in groups:
                    # Seed PSUM with D*u via diag(D) matmuls
                    ps_t = {}
                    for ct in grp:
                        for h in range(2):
                            ps = psp.tile([128, half], F32,
                                          tag=f"ps{ct % CT_GROUP}_{h}",
                                          name=f"ps{ct}_{q}_{h}")
                            ps_t[(ct, h)] = ps
                            cols = slice(h * half, (h + 1) * half)
                            nc.tensor.matmul(ps[:], diag_sb[ct][:],
                                             u_all[ct][:, cols],
                                             start=True, stop=False)
                    for n in range(dstate):
                        Bbc = bcp.tile([128, s_chunk], BF16, tag="Bbc",
                                       name=f"Bbc{n}_{q}_{grp[0]}")
                        nc.sync.dma_start(
                            Bbc[:],
                            B_d[n:n + 1, lo:hi].broadcast_to([128, s_chunk]))
                        Cbc = bcp.tile([128, s_chunk], BF16, tag="Cbc",
                                       name=f"Cbc{n}_{q}_{grp[0]}")
                        nc.sync.dma_start(
                            Cbc[:],
                            C_d[n:n + 1, lo:hi].broadcast_to([128, s_chunk]))
                        for ct in grp:
                            dA = work.tile([128, s_chunk], F32, tag="dA",
                                           name=f"dA{n}_{ct}_{q}")
                            nc.scalar.activation(dA[:], dt_all[ct][:],
                                                 ACTF.Exp, bias=0.0,
                                                 scale=A_sb[ct][:, n:n + 1])
                            dBu = work.tile([128, s_chunk], BF16, tag="dBu",
                                            name=f"dBu{n}_{ct}_{q}")
                            nc.vector.tensor_tensor(dBu[:], dtu_all[ct][:],
                                                    Bbc[:], op=ALU.mult)
                            x = work.tile([128, s_chunk], BF16, tag="x",
                                          name=f"x{n}_{ct}_{q}")
                            init = (0.0 if q == 0
                                    else states[ct][:, n:n + 1])
                            nc.vector.tensor_tensor_scan(x[:], dA[:], dBu[:],
                                                         init, op0=ALU.mult,
                                                         op1=ALU.add)
                            if q < n_q - 1:
                                nc.vector.tensor_copy(
                                    states[ct][:, n:n + 1],
                                    x[:, s_chunk - 1:s_chunk])
                            xC = work.tile([128, s_chunk], BF16, tag="xC",
                                           name=f"xC{n}_{ct}_{q}")
                            nc.vector.tensor_tensor(xC[:], x[:], Cbc[:],
                                                    op=ALU.mult)
                            last = n == dstate - 1
                            for h in range(2):
                                cols = slice(h * half, (h + 1) * half)
                                nc.tensor.matmul(ps_t[(ct, h)][:], eyeb[:],
                                                 xC[:, cols], start=False,
                                                 stop=last)
                    # Evict PSUM -> SBUF -> DRAM
                    for ct in grp:
                        rows = slice(ct * 128, (ct + 1) * 128)
                        yo = yout.tile([128, s_chunk], F32, tag="yo",
                                       name=f"yo{ct}_{q}")
                        for h in range(2):
                            cols = slice(h * half, (h + 1) * half)
                            nc.scalar.copy(yo[:, cols], ps_t[(ct, h)][:])
                        nc.sync.dma_start(y_d[rows, lo:hi], yo[:])

    nc.compile()
    return nc


_NC_CACHE = {}


def _get_nc():
    if "nc" not in _NC_CACHE:
        _NC_CACHE["nc"] = build_nc()
    return _NC_CACHE["nc"]


def shard_inputs(u, delta, B, C, A, D, delta_bias):
    eye = np.eye(128, dtype=np.float32)
    in_maps = []
    for c in range(N_CORES):
        b, h = divmod(c, 2)
        sl = slice(h * DL, (h + 1) * DL)
        in_maps.append({
            "u": np.ascontiguousarray(u[b, sl]),
            "delta": np.ascontiguousarray(delta[b, sl]),
            "Bmat": np.ascontiguousarray(B[b]).astype(ml_dtypes.bfloat16),
            "Cmat": np.ascontiguousarray(C[b]).astype(ml_dtypes.bfloat16),
            "Amat": np.ascontiguousarray(A[sl]),
            "Dvec": np.ascontiguousarray(D[sl]).reshape(DL, 1),
            "dbias": np.ascontiguousarray(delta_bias[sl]).reshape(DL, 1),
            "eye": eye,
        })
    return in_maps


def kernel(u, delta, B, C, A, D, delta_bias, trace=False):
    u = np.asarray(u, np.float32)
    delta = np.asarray(delta, np.float32)
    B = np.asarray(B, np.float32)
    C = np.asarray(C, np.float32)
    A = np.asarray(A, np.float32)
    D = np.asarray(D, np.float32)
    delta_bias = np.asarray(delta_bias, np.float32)

    nc = _get_nc()
    in_maps = shard_inputs(u, delta, B, C, A, D, delta_bias)
    res = run_bass_kernel_spmd(nc, in_maps, core_ids=list(range(N_CORES)),
                               trace=trace)
    y = np.empty((BATCH, DIM, SEQ), np.float32)
    for c, r in enumerate(res.results):
        b, h = divmod(c, 2)
        y[b, h * DL:(h + 1) * DL] = r["y"]
    if trace:
        return y, res
    return y
